# revision 11
# baseline (speedup 1.0000x reference)
"""ChemConv (GNN message passing) kernel for Trainium2, 8 NeuronCores.

Reference math (per sample s):
    node_conn[a,f,d] = sum_n conn[a,n,f] * node[n,d]
    out[a,o]         = sum_{f,d} cat(node_conn, bond)[a,f,d] * filters[o,f,d]

Folded form used on device (filters folded into node features):
    W[n,f,o]  = sum_d node[n,d] * filters[o,f,d]          (tiny matmuls)
    out[a,o]  = sum_{n,f} conn[a,(n,f)] * W[(n,f),o]
              + sum_{f,j} bond[a,f,j] * filters[o,f,64+j]

Sharding: data-parallel over the 32 samples -> 4 samples per core x 8 cores.

DMA layout trick: the PE contracts over the partition dim, so conn must land
with n on partitions; conn's HBM layout is [a, n, f] with f (12 floats = 48 B)
innermost.  Splitting n = 4*nh + nl and loading tiles
[nh=128 partitions, (a, nl, f)] gives 192-byte contiguous runs per (partition,
atom) - 4x larger DMA descriptors than the naive n-on-partitions load - while
keeping the full contraction reachable via 4 accumulating matmuls (one per nl)
with W's rows permuted to match (computed on chip, so its layout is free).

Walrus quirk: a Matmult instruction (its LDWEIGHTS struct) can carry at most
ONE semaphore wait; Tile freely attaches several.  After Tile scheduling we
hoist the extra waits onto NoOps inserted directly before the matmul on the
same engine queue - semantically identical (waits execute in queue order).
"""

import sys

import numpy as np

try:
    import concourse.bass as bass
except ImportError:  # pragma: no cover
    sys.path.append("/opt/trn_rl_repo")
    import concourse.bass as bass

import concourse.mybir as mybir
from concourse import bass_utils
from concourse.tile import TileContext

N_SAMPLES, N_ATOMS = 32, 512
IN_DEPTH, OUT_DEPTH, FL = 64, 64, 12
N_CORES = 8
S_PER_CORE = N_SAMPLES // N_CORES  # 4

NL = 4  # low bits of n folded into the free dim (192-byte DMA runs)
NH = N_ATOMS // NL  # 128 partitions
A_BLK = 256  # atoms per connectivity tile
N_ABLK = N_ATOMS // A_BLK

PSW_BUFS = 4  # psum banks for W building
PO_BUFS = 4  # psum banks for output accumulation

_DT = mybir.dt.float32


def _build_bass(repeat=1):
    """repeat > 1 re-runs phase 2 (the C stream + matmuls) that many times
    inside the NEFF - output is identical; used only to amortize the host
    dispatch overhead when measuring device-side time."""
    nc = bass.Bass()
    c = nc.dram_tensor(
        "c", (S_PER_CORE, N_ATOMS, N_ATOMS, FL), _DT, kind="ExternalInput"
    )
    xt = nc.dram_tensor(
        "xt", (S_PER_CORE, IN_DEPTH, N_ATOMS), _DT, kind="ExternalInput"
    )
    bt = nc.dram_tensor("bt", (S_PER_CORE, 2 * FL, N_ATOMS), _DT, kind="ExternalInput")
    ftf = nc.dram_tensor("ftf", (IN_DEPTH, FL * OUT_DEPTH), _DT, kind="ExternalInput")
    fb = nc.dram_tensor("fb", (2 * FL, OUT_DEPTH), _DT, kind="ExternalInput")
    out_t = nc.dram_tensor(
        "out_t", (S_PER_CORE, OUT_DEPTH, N_ATOMS), _DT, kind="ExternalOutput"
    )

    HALF = FL * OUT_DEPTH // 2  # 384 columns per W-build matmul (one psum bank)

    with TileContext(nc) as tc:
        with (
            tc.tile_pool(name="consts", bufs=1) as consts,
            tc.tile_pool(name="cpool", bufs=2) as cpool,
            tc.tile_pool(name="wpool", bufs=S_PER_CORE) as wpool,
            tc.tile_pool(name="small", bufs=S_PER_CORE) as small,
            tc.tile_pool(name="outp", bufs=3) as outp,
            tc.tile_pool(name="psum", bufs=PO_BUFS, space="PSUM") as psum,
            tc.tile_pool(name="psumw", bufs=PSW_BUFS, space="PSUM") as psumw,
        ):
            ftf_sb = consts.tile([IN_DEPTH, FL * OUT_DEPTH], _DT)
            ftf_dma = nc.sync.dma_start(out=ftf_sb[:], in_=ftf[:])
            fb_sb = consts.tile([2 * FL, OUT_DEPTH], _DT)
            fb_dma = nc.sync.dma_start(out=fb_sb[:], in_=fb[:])

            xt_sbs, bt_sbs, xt_dmas, bt_dmas = [], [], [], []
            for s in range(S_PER_CORE):
                xt_sb = small.tile([IN_DEPTH, N_ATOMS], _DT, tag="xt")
                xt_dmas.append(nc.sync.dma_start(out=xt_sb[:], in_=xt[s]))
                xt_sbs.append(xt_sb)
                bt_sb = small.tile([2 * FL, N_ATOMS], _DT, tag="bt")
                bt_dmas.append(nc.sync.dma_start(out=bt_sb[:], in_=bt[s]))
                bt_sbs.append(bt_sb)

            # ---- Phase 1: W for all samples --------------------------------
            # W[nh, nl, f, o] = sum_d node[4*nh + nl, d] * filters[o, f, d]
            w_sbs = []
            for s in range(S_PER_CORE):
                w_sb = wpool.tile([NH, NL, FL, OUT_DEPTH], _DT, tag="w")
                w_sbs.append(w_sb)
                for j in range(NL):
                    for h in range(2):
                        pw = psumw.tile([NH, FL // 2, OUT_DEPTH], _DT, tag="pw")
                        nc.tensor.matmul(
                            pw[:],
                            lhsT=xt_sbs[s][:, j::NL],  # cols are n = 4*nh+j
                            rhs=ftf_sb[:, h * HALF : (h + 1) * HALF],
                            start=True,
                            stop=True,
                        )
                        nc.vector.tensor_copy(
                            out=w_sb[:, j, h * (FL // 2) : (h + 1) * (FL // 2), :],
                            in_=pw[:],
                        )

            # ---- Phase 2: stream connectivity, accumulate output -----------
            for s in [s for _ in range(repeat) for s in range(S_PER_CORE)]:
                for ab in range(N_ABLK):
                    ct = cpool.tile([NH, A_BLK, NL, FL], _DT, tag="ct")
                    nc.sync.dma_start(
                        out=ct[:],
                        in_=c[s, ab * A_BLK : (ab + 1) * A_BLK].rearrange(
                            "a (nh nl) f -> nh a nl f", nl=NL
                        ),
                    )

                    po = psum.tile([OUT_DEPTH, A_BLK], _DT, tag="po")
                    # bond contribution first: out[o,a] += fb[fj,o]^T @ bt[fj,a]
                    nc.tensor.matmul(
                        po[:],
                        lhsT=fb_sb[:],
                        rhs=bt_sbs[s][:, ab * A_BLK : (ab + 1) * A_BLK],
                        start=True,
                        stop=False,
                    )
                    for f in range(FL):
                        for j in range(NL):
                            nc.tensor.matmul(
                                po[:],
                                lhsT=w_sbs[s][:, j, f, :],  # [128, 64]
                                rhs=ct[:, :, j, f],  # [128, A_BLK]
                                start=False,
                                stop=(f == FL - 1 and j == NL - 1),
                            )
                    ot = outp.tile([OUT_DEPTH, A_BLK], _DT, tag="ot")
                    nc.vector.tensor_copy(out=ot[:], in_=po[:])
                    nc.sync.dma_start(
                        out=out_t[s, :, ab * A_BLK : (ab + 1) * A_BLK], in_=ot[:]
                    )

    _hoist_extra_waits(nc)
    return nc


def _hoist_extra_waits(nc):
    """This walrus build rejects any instruction struct carrying more than one
    semaphore wait ("Too many sync wait commands"); Tile freely attaches
    several.  Waits execute in engine-queue order, so hoisting the extras onto
    NoOps inserted directly before the instruction on the same engine is
    semantically identical.  One wait per NoOp."""
    for f in nc.m.functions:
        for blk in f.blocks:
            insts = blk.instructions
            rebuilt = []
            changed = False
            for inst in insts:
                si = inst.sync_info
                if si is not None and len(si.on_wait) > 1:
                    for w in list(si.on_wait)[:-1]:
                        nop = mybir.InstNoOp(
                            name=nc.get_next_instruction_name(),
                            engine=inst.engine,
                            ins=[],
                            outs=[],
                            sync_info=mybir.SyncInfo(on_wait=[w], on_update=[]),
                        )
                        nc.register_instruction(nop)
                        rebuilt.append(nop)
                    inst.sync_info = mybir.SyncInfo(
                        on_wait=[list(si.on_wait)[-1]], on_update=list(si.on_update)
                    )
                    changed = True
                rebuilt.append(inst)
            if changed:
                del insts[:]
                insts.extend(rebuilt)


_CACHED_NC = {}


def _get_nc(repeat=1):
    if repeat not in _CACHED_NC:
        _CACHED_NC[repeat] = _build_bass(repeat)
    return _CACHED_NC[repeat]


def _prep_inputs(node, conn, bond, filters):
    """Host-side layout-only prep + per-core sharding."""
    node = np.asarray(node, dtype=np.float32)
    conn = np.asarray(conn, dtype=np.float32)
    bond = np.asarray(bond, dtype=np.float32)
    filters = np.asarray(filters, dtype=np.float32)

    # xt[s, d, n] = node[s, n, d]
    node_t = np.ascontiguousarray(node.transpose(0, 2, 1))
    # bt[s, f*2+j, a] = bond[s, a, f, j]
    bond_t = np.ascontiguousarray(bond.transpose(0, 2, 3, 1)).reshape(
        N_SAMPLES, 2 * FL, N_ATOMS
    )
    # ftf[d, f*64+o] = filters[o, f, d]
    ftf = np.ascontiguousarray(filters[:, :, :IN_DEPTH].transpose(2, 1, 0)).reshape(
        IN_DEPTH, FL * OUT_DEPTH
    )
    # fb[f*2+j, o] = filters[o, f, 64+j]
    fb = np.ascontiguousarray(filters[:, :, IN_DEPTH:].transpose(1, 2, 0)).reshape(
        2 * FL, OUT_DEPTH
    )

    in_maps = []
    for k in range(N_CORES):
        s0 = k * S_PER_CORE
        in_maps.append(
            {
                "c": np.ascontiguousarray(conn[s0 : s0 + S_PER_CORE]),
                "xt": np.ascontiguousarray(node_t[s0 : s0 + S_PER_CORE]),
                "bt": np.ascontiguousarray(bond_t[s0 : s0 + S_PER_CORE]),
                "ftf": ftf,
                "fb": fb,
            }
        )
    return in_maps


def run(node_property_tensor, connectivity_tensor, bond_property_tensor, filters,
        trace=False):
    """Run on 8 cores; returns (output [32,512,64], BassKernelResults)."""
    nc = _get_nc()
    in_maps = _prep_inputs(
        node_property_tensor, connectivity_tensor, bond_property_tensor, filters
    )
    res = bass_utils.run_bass_kernel_spmd(
        nc, in_maps, core_ids=list(range(N_CORES)), trace=trace
    )
    out = np.empty((N_SAMPLES, N_ATOMS, OUT_DEPTH), dtype=np.float32)
    for k in range(N_CORES):
        s0 = k * S_PER_CORE
        out[s0 : s0 + S_PER_CORE] = res.results[k]["out_t"].transpose(0, 2, 1)
    return out, res


def kernel(node_property_tensor, connectivity_tensor, bond_property_tensor, filters):
    out, _ = run(
        node_property_tensor, connectivity_tensor, bond_property_tensor, filters
    )
    return out
